# revision 9
# baseline (speedup 1.0000x reference)
"""Masked dot-product attention on 8 Trainium2 NeuronCores (Bass/Tile).

Problem: queries/keys/values [32, 1024, 128] f32, valid_lens [32] i32.
  out = softmax(mask(Q K^T / sqrt(128))) V        (key-padding prefix mask)

Strategy (batch-parallel, 4 batches per core, one SPMD program):
  * Host pre-transposes Q and K per batch to [D=128, 1024] so the
    contraction dim D sits on SBUF partitions; no on-device transposes.
  * Scores are computed transposed: S^T[k, q] = (K^T chunk).T @ Q^T with k
    in chunks of 128 partitions.
  * The prefix key mask is per-PARTITION in this layout, so it folds into
    the exp for free: ACT computes exp(S^T * 1/sqrt(D) + bias) with
    bias[k] in {0, -1e6}; masked rows become exactly 0.
  * out^T[v, q] += V_chunk-as-lhsT @ expS^T accumulates in PSUM across
    k chunks (V is loaded chunk-major, no transpose needed).
  * denominator: expS^T chunks are summed in SBUF on DVE, then one
    ones-column matmul reduces across partitions -> sums[1, q].
  * normalize: recip = 1/sums (DVE), broadcast across partitions via a
    K=1 matmul with a ones row, one DVE multiply -> SBUF -> DMA out.
  * Host transposes out^T back to [q, v] while gathering.

Static masked-chunk skipping: batch b only needs ceil(valid_lens[b]/128)
key chunks; the rest contribute exactly 0. Batches are assigned to the 4
per-core slots by descending need (sorted, slot-major), so slot j's
compile-time chunk count is max over its 8 batches. The SPMD program is
specialized to that profile at kernel build time.

All inputs of one batch are packed into a single DRAM tensor/DMA so that
downstream instructions depend on one DMA semaphore (walrus allows only
2 sync waits per Matmult).
"""

import math

import numpy as np

import concourse.bacc as bacc
import concourse.bass as bass
import concourse.mybir as mybir
import concourse.tile as tile
from concourse.bass_utils import run_bass_kernel_spmd

B, Q, K, D = 32, 1024, 1024, 128
N_CORES = 8
BPC = B // N_CORES  # batches per core
PART = 128          # partition size / key chunk size
NCHUNK = K // PART
MASK_BIAS = -1.0e6
INV_SQRT_D = 1.0 / math.sqrt(D)
F32 = mybir.dt.float32

_NC_CACHE: dict = {}


def _pack_cols(cap: int):
    """Column layout of the packed per-batch input: qt | kt | vp."""
    kcols = cap * PART
    q0 = 0
    k0 = q0 + Q
    v0 = k0 + kcols
    total = v0 + kcols
    return q0, k0, v0, total


def build_nc(profile: tuple) -> bass.Bass:
    """Build the SPMD Bass program for a per-slot chunk-count profile."""
    nc = bacc.Bacc()
    ins = [
        nc.declare_dram_parameter(
            f"in{s}", [PART, _pack_cols(profile[s])[-1]], F32, isOutput=False
        )
        for s in range(BPC)
    ]
    mb = nc.declare_dram_parameter("mb", [PART, BPC * NCHUNK], F32, isOutput=False)
    out = nc.declare_dram_parameter("out", [BPC, PART, Q], F32, isOutput=True)

    with tile.TileContext(nc) as tc:
        with (
            tc.tile_pool(name="io", bufs=2) as io,
            tc.tile_pool(name="probs", bufs=3) as probs,
            tc.tile_pool(name="consts", bufs=1) as consts,
            tc.tile_pool(name="ps_s", bufs=2, space="PSUM") as ps_s,
            tc.tile_pool(name="ps_out", bufs=1, space="PSUM") as ps_out,
            tc.tile_pool(name="ps_sum", bufs=1, space="PSUM") as ps_sum,
        ):
            ones_col = consts.tile([PART, 1], F32)  # lhsT for the denominator
            nc.vector.memset(ones_col, 1.0)
            ones_row = consts.tile([1, PART], F32)  # lhsT for recip broadcast
            nc.vector.memset(ones_row, 1.0)
            mb_sb = consts.tile([PART, BPC * NCHUNK], F32)
            nc.sync.dma_start(out=mb_sb, in_=mb[:, :])

            for b in range(BPC):
                cap = profile[b]
                q0, k0, v0, total = _pack_cols(cap)

                in_sb = io.tile([PART, total], F32, tag=f"in{b}", bufs=1)
                nc.sync.dma_start(out=in_sb, in_=ins[b][:, :])
                qt_sb = in_sb[:, q0:q0 + Q]
                out_ps = ps_out.tile([PART, Q], F32, tag="out")
                sums_ps = ps_sum.tile([1, Q], F32, tag="sums")

                for c in range(cap):
                    s_ps = ps_s.tile([PART, Q], F32, tag="s")
                    kw = in_sb[:, k0 + c * PART:k0 + (c + 1) * PART]
                    for h in range(2):
                        nc.tensor.matmul(
                            s_ps[:, h * 512:(h + 1) * 512],
                            kw,
                            qt_sb[:, h * 512:(h + 1) * 512],
                            start=True,
                            stop=True,
                        )
                    p_sb = probs.tile([PART, Q], F32, tag="p")
                    nc.scalar.activation(
                        p_sb,
                        s_ps,
                        mybir.ActivationFunctionType.Exp,
                        bias=mb_sb[:, b * NCHUNK + c:b * NCHUNK + c + 1],
                        scale=INV_SQRT_D,
                    )
                    vw = in_sb[:, v0 + c * PART:v0 + (c + 1) * PART]
                    first, last = c == 0, c == cap - 1
                    for h in range(2):
                        nc.tensor.matmul(
                            out_ps[:, h * 512:(h + 1) * 512],
                            vw,
                            p_sb[:, h * 512:(h + 1) * 512],
                            start=first,
                            stop=last,
                        )
                    for h in range(2):
                        nc.tensor.matmul(
                            sums_ps[:, h * 512:(h + 1) * 512],
                            ones_col,
                            p_sb[:, h * 512:(h + 1) * 512],
                            start=first,
                            stop=last,
                        )

                # Epilogue: reciprocal, broadcast, normalize, store.
                recip_sb = probs.tile([1, Q], F32, tag="recip")
                nc.vector.reciprocal(recip_sb, sums_ps)
                rb_ps = ps_s.tile([PART, Q], F32, tag="s")
                for h in range(2):
                    nc.tensor.matmul(
                        rb_ps[:, h * 512:(h + 1) * 512],
                        ones_row,
                        recip_sb[:, h * 512:(h + 1) * 512],
                        start=True,
                        stop=True,
                    )
                rb_sb = probs.tile([PART, Q], F32, tag="rb")
                nc.vector.tensor_copy(rb_sb, rb_ps)
                outn = io.tile([PART, Q], F32, tag="outn")
                nc.vector.tensor_mul(outn, out_ps, rb_sb)
                nc.sync.dma_start(out=out[b], in_=outn)

    nc.compile()
    return nc


def plan(valid_lens: np.ndarray):
    """Assign batches to (core, slot) and derive the chunk-count profile.

    Sorting by descending need and slicing slot-major minimizes the sum of
    per-slot maxima, which is the per-core static work.
    """
    need = np.minimum((valid_lens.astype(np.int64) + PART - 1) // PART, NCHUNK)
    need = np.maximum(need, 1)
    order = np.argsort(-need, kind="stable")
    perm = order.reshape(BPC, N_CORES)  # perm[slot, core] = batch index
    profile = tuple(int(need[perm[s]].max()) for s in range(BPC))
    return perm, profile


def kernel(queries, keys, values, valid_lens):
    q = np.ascontiguousarray(np.asarray(queries, dtype=np.float32))
    k = np.ascontiguousarray(np.asarray(keys, dtype=np.float32))
    v = np.ascontiguousarray(np.asarray(values, dtype=np.float32))
    lens = np.asarray(valid_lens).astype(np.int64).reshape(B)

    perm, profile = plan(lens)

    if profile not in _NC_CACHE:
        _NC_CACHE[profile] = build_nc(profile)
    nc = _NC_CACHE[profile]

    arange_k = np.arange(K)
    in_maps = []
    for core in range(N_CORES):
        im = {}
        mb_h = np.empty((PART, BPC * NCHUNK), np.float32)
        for slot in range(BPC):
            cap = profile[slot]
            q0, k0, v0, total = _pack_cols(cap)
            kcols = cap * PART
            bidx = int(perm[slot, core])
            buf = np.empty((PART, total), np.float32)
            buf[:, q0:q0 + Q] = q[bidx].T
            buf[:, k0:k0 + kcols] = k[bidx, :kcols].T
            # v chunk-major: vp[p, c*128 + d] = v[c*128 + p, d]
            buf[:, v0:v0 + kcols] = (
                v[bidx, :kcols]
                .reshape(cap, PART, D)
                .transpose(1, 0, 2)
                .reshape(PART, kcols)
            )
            im[f"in{slot}"] = buf
            # bias[p, c] = 0 if (c*128+p) < L else -1e6
            valid = (arange_k < lens[bidx]).reshape(NCHUNK, PART).T  # [p, c]
            mb_h[:, slot * NCHUNK:(slot + 1) * NCHUNK] = np.where(
                valid, 0.0, MASK_BIAS
            )
        im["mb"] = mb_h
        in_maps.append(im)

    res = run_bass_kernel_spmd(nc, in_maps, list(range(N_CORES)))

    out = np.empty((B, Q, D), np.float32)
    for core in range(N_CORES):
        core_out = res.results[core]["out"]  # [BPC, 128(v), 1024(q)]
        for slot in range(BPC):
            bidx = int(perm[slot, core])
            out[bidx] = core_out[slot].T
    return out


# revision 15
# speedup vs baseline: 1.7924x; 1.7924x over previous
"""Masked dot-product attention on 8 Trainium2 NeuronCores (Bass/Tile).

Problem: queries/keys/values [32, 1024, 128] f32, valid_lens [32] i32.
  out = softmax(mask(Q K^T / sqrt(128))) V        (key-padding prefix mask)

Strategy (batch-parallel, 4 batches per core, one SPMD program):
  * Host pre-transposes Q and K per batch to [D=128, 1024] so the
    contraction dim D sits on SBUF partitions; no on-device transposes.
  * Scores are computed transposed: S^T[k, q] = (K^T chunk).T @ Q^T with k
    in chunks of 128 partitions.
  * The prefix key mask is per-PARTITION in this layout, so it folds into
    the exp for free: ACT computes exp(S^T * 1/sqrt(D) + bias) with
    bias[k] in {0, -1e6}; masked rows become exactly 0.
  * out^T[v, q] += V_chunk-as-lhsT @ expS^T accumulates in PSUM across
    k chunks (V is loaded chunk-major, no transpose needed).
  * denominator: expS^T chunks are summed in SBUF on DVE, then one
    ones-column matmul reduces across partitions -> sums[1, q].
  * normalize: recip = 1/sums (DVE), broadcast across partitions via a
    K=1 matmul with a ones row, one DVE multiply -> SBUF -> DMA out.
  * Host transposes out^T back to [q, v] while gathering.

Static masked-chunk skipping: batch b only needs ceil(valid_lens[b]/128)
key chunks; the rest contribute exactly 0. Batches are assigned to the 4
per-core slots by descending need (sorted, slot-major), so slot j's
compile-time chunk count is max over its 8 batches. The SPMD program is
specialized to that profile at kernel build time.

All inputs of one batch are packed into a single DRAM tensor/DMA so that
downstream instructions depend on one DMA semaphore (walrus allows only
2 sync waits per Matmult).
"""

import math

import numpy as np

import concourse.bacc as bacc
import concourse.bass as bass
import concourse.mybir as mybir
import concourse.tile as tile
from concourse.bass_utils import run_bass_kernel_spmd

B, Q, K, D = 32, 1024, 1024, 128
N_CORES = 8
BPC = B // N_CORES  # batches per core
PART = 128          # partition size / key chunk size
NCHUNK = K // PART
MASK_BIAS = -1.0e6
INV_SQRT_D = 1.0 / math.sqrt(D)
F32 = mybir.dt.float32
F32R = mybir.dt.float32r

# fp32 matmuls stream at 4 cycles/row on the PE; float32r streams at 1
# cycle/row (N>=256) with a reduced-precision multiply. Accuracy was
# validated empirically on HW against the fp32 jax reference.
import os as _os

R_SCORES = _os.environ.get("ATTN_R_SCORES", "1") == "1"
R_AV = _os.environ.get("ATTN_R_AV", "1") == "1"
R_SUMS = _os.environ.get("ATTN_R_SUMS", "1") == "1"
R_RB = _os.environ.get("ATTN_R_RB", "1") == "1"


def _mm(nc, out, lhsT, rhs, fast, **kw):
    if not fast:
        lhsT = lhsT.bitcast(F32)
        rhs = rhs.bitcast(F32)
    nc.tensor.matmul(out, lhsT, rhs, **kw)

_NC_CACHE: dict = {}


def _pack_cols(cap: int):
    """Column layout of the packed per-batch input: qt | kt | vp."""
    kcols = cap * PART
    q0 = 0
    k0 = q0 + Q
    v0 = k0 + kcols
    total = v0 + kcols
    return q0, k0, v0, total


def build_nc(profile: tuple) -> bass.Bass:
    """Build the SPMD Bass program for a per-slot chunk-count profile."""
    nc = bacc.Bacc()
    ins = [
        nc.declare_dram_parameter(
            f"in{s}", [PART, _pack_cols(profile[s])[-1]], F32R, isOutput=False
        )
        for s in range(BPC)
    ]
    mb = nc.declare_dram_parameter("mb", [PART, BPC * NCHUNK], F32, isOutput=False)
    cst = nc.declare_dram_parameter("cst", [PART, PART + 1], F32R, isOutput=False)
    out = nc.declare_dram_parameter("out", [BPC, PART, Q], F32, isOutput=True)

    with tile.TileContext(nc) as tc:
        with (
            tc.tile_pool(name="io", bufs=2) as io,
            tc.tile_pool(name="probs", bufs=3) as probs,
            tc.tile_pool(name="consts", bufs=1) as consts,
            tc.tile_pool(name="ps_s", bufs=2, space="PSUM") as ps_s,
            tc.tile_pool(name="ps_out", bufs=1, space="PSUM") as ps_out,
            tc.tile_pool(name="ps_sum", bufs=1, space="PSUM") as ps_sum,
        ):
            cst_sb = consts.tile([PART, PART + 1], F32R)
            nc.sync.dma_start(out=cst_sb, in_=cst[:, :])
            ones_col = cst_sb[:, 0:1]    # lhsT for the denominator
            ones_row = cst_sb[0:1, 1:PART + 1]  # lhsT for recip broadcast
            mb_sb = consts.tile([PART, BPC * NCHUNK], F32)
            nc.sync.dma_start(out=mb_sb, in_=mb[:, :])

            for b in range(BPC):
                cap = profile[b]
                q0, k0, v0, total = _pack_cols(cap)

                in_sb = io.tile([PART, total], F32R, tag=f"in{b}", bufs=1)
                nc.sync.dma_start(out=in_sb, in_=ins[b][:, :])
                qt_sb = in_sb[:, q0:q0 + Q]
                out_ps = ps_out.tile([PART, Q], F32, tag="out")
                sums_ps = ps_sum.tile([1, Q], F32, tag="sums")

                for c in range(cap):
                    s_ps = ps_s.tile([PART, Q], F32, tag="s")
                    kw = in_sb[:, k0 + c * PART:k0 + (c + 1) * PART]
                    for h in range(2):
                        _mm(
                            nc,
                            s_ps[:, h * 512:(h + 1) * 512],
                            kw,
                            qt_sb[:, h * 512:(h + 1) * 512],
                            R_SCORES,
                            start=True,
                            stop=True,
                        )
                    p_sb = probs.tile([PART, Q], F32R, tag="p")
                    nc.scalar.activation(
                        p_sb,
                        s_ps,
                        mybir.ActivationFunctionType.Exp,
                        bias=mb_sb[:, b * NCHUNK + c:b * NCHUNK + c + 1],
                        scale=INV_SQRT_D,
                    )
                    vw = in_sb[:, v0 + c * PART:v0 + (c + 1) * PART]
                    first, last = c == 0, c == cap - 1
                    for h in range(2):
                        _mm(
                            nc,
                            out_ps[:, h * 512:(h + 1) * 512],
                            vw,
                            p_sb[:, h * 512:(h + 1) * 512],
                            R_AV,
                            start=first,
                            stop=last,
                        )
                    for h in range(2):
                        _mm(
                            nc,
                            sums_ps[:, h * 512:(h + 1) * 512],
                            ones_col,
                            p_sb[:, h * 512:(h + 1) * 512],
                            R_SUMS,
                            start=first,
                            stop=last,
                        )

                # Epilogue: reciprocal, broadcast, normalize, store.
                recip_sb = probs.tile([1, Q], F32R, tag="recip")
                with nc.allow_low_precision(reason="fp32r recip feeds fp32r bcast matmul"):
                    nc.vector.reciprocal(recip_sb, sums_ps)
                rb_ps = ps_s.tile([PART, Q], F32, tag="s")
                for h in range(2):
                    _mm(
                        nc,
                        rb_ps[:, h * 512:(h + 1) * 512],
                        ones_row,
                        recip_sb[:, h * 512:(h + 1) * 512],
                        R_RB,
                        start=True,
                        stop=True,
                    )
                rb_sb = probs.tile([PART, Q], F32, tag="rb")
                nc.vector.tensor_copy(rb_sb, rb_ps)
                outn = io.tile([PART, Q], F32, tag="outn")
                nc.vector.tensor_mul(outn, out_ps, rb_sb)
                nc.sync.dma_start(out=out[b], in_=outn)

    nc.compile()
    return nc


def plan(valid_lens: np.ndarray):
    """Assign batches to (core, slot) and derive the chunk-count profile.

    Sorting by descending need and slicing slot-major minimizes the sum of
    per-slot maxima, which is the per-core static work.
    """
    need = np.minimum((valid_lens.astype(np.int64) + PART - 1) // PART, NCHUNK)
    need = np.maximum(need, 1)
    order = np.argsort(-need, kind="stable")
    perm = order.reshape(BPC, N_CORES)  # perm[slot, core] = batch index
    profile = tuple(int(need[perm[s]].max()) for s in range(BPC))
    return perm, profile


def kernel(queries, keys, values, valid_lens):
    q = np.ascontiguousarray(np.asarray(queries, dtype=np.float32))
    k = np.ascontiguousarray(np.asarray(keys, dtype=np.float32))
    v = np.ascontiguousarray(np.asarray(values, dtype=np.float32))
    lens = np.asarray(valid_lens).astype(np.int64).reshape(B)

    perm, profile = plan(lens)

    if profile not in _NC_CACHE:
        _NC_CACHE[profile] = build_nc(profile)
    nc = _NC_CACHE[profile]

    arange_k = np.arange(K)
    in_maps = []
    for core in range(N_CORES):
        im = {}
        mb_h = np.empty((PART, BPC * NCHUNK), np.float32)
        for slot in range(BPC):
            cap = profile[slot]
            q0, k0, v0, total = _pack_cols(cap)
            kcols = cap * PART
            bidx = int(perm[slot, core])
            buf = np.empty((PART, total), np.float32)
            buf[:, q0:q0 + Q] = q[bidx].T
            buf[:, k0:k0 + kcols] = k[bidx, :kcols].T
            # v chunk-major: vp[p, c*128 + d] = v[c*128 + p, d]
            buf[:, v0:v0 + kcols] = (
                v[bidx, :kcols]
                .reshape(cap, PART, D)
                .transpose(1, 0, 2)
                .reshape(PART, kcols)
            )
            im[f"in{slot}"] = buf
            # bias[p, c] = 0 if (c*128+p) < L else -1e6
            valid = (arange_k < lens[bidx]).reshape(NCHUNK, PART).T  # [p, c]
            mb_h[:, slot * NCHUNK:(slot + 1) * NCHUNK] = np.where(
                valid, 0.0, MASK_BIAS
            )
        im["mb"] = mb_h
        im["cst"] = np.ones((PART, PART + 1), np.float32)
        in_maps.append(im)

    res = run_bass_kernel_spmd(nc, in_maps, list(range(N_CORES)))

    out = np.empty((B, Q, D), np.float32)
    for core in range(N_CORES):
        core_out = res.results[core]["out"]  # [BPC, 128(v), 1024(q)]
        for slot in range(BPC):
            bidx = int(perm[slot, core])
            out[bidx] = core_out[slot].T
    return out


# revision 17
# speedup vs baseline: 2.8602x; 1.5957x over previous
"""Masked dot-product attention on 8 Trainium2 NeuronCores (Bass/Tile).

Problem: queries/keys/values [32, 1024, 128] f32, valid_lens [32] i32.
  out = softmax(mask(Q K^T / sqrt(128))) V        (key-padding prefix mask)

Strategy (batch-parallel, 4 batches per core, one SPMD program):
  * Host pre-transposes Q and K per batch to [D=128, 1024] so the
    contraction dim D sits on SBUF partitions; no on-device transposes.
  * Scores are computed transposed: S^T[k, q] = (K^T chunk).T @ Q^T with k
    in chunks of 128 partitions.
  * The prefix key mask is per-PARTITION in this layout, so it folds into
    the exp for free: ACT computes exp(S^T * 1/sqrt(D) + bias) with
    bias[k] in {0, -1e6}; masked rows become exactly 0.
  * out^T[v, q] += V_chunk-as-lhsT @ expS^T accumulates in PSUM across
    k chunks (V is loaded chunk-major, no transpose needed).
  * denominator: expS^T chunks are summed in SBUF on DVE, then one
    ones-column matmul reduces across partitions -> sums[1, q].
  * out^T and sums are DMA'd back; the host divides and transposes
    while gathering (0.003% of the FLOPs).

Static masked-chunk skipping: batch b only needs ceil(valid_lens[b]/128)
key chunks; the rest contribute exactly 0. Batches are assigned to the 4
per-core slots by descending need (sorted, slot-major), so slot j's
compile-time chunk count is max over its 8 batches. The SPMD program is
specialized to that profile at kernel build time.

All inputs of one batch are packed into a single DRAM tensor/DMA so that
downstream instructions depend on one DMA semaphore (walrus allows only
2 sync waits per Matmult).
"""

import math

import numpy as np

import concourse.bacc as bacc
import concourse.bass as bass
import concourse.mybir as mybir
import concourse.tile as tile
from concourse.bass_utils import run_bass_kernel_spmd

B, Q, K, D = 32, 1024, 1024, 128
N_CORES = 8
BPC = B // N_CORES  # batches per core
PART = 128          # partition size / key chunk size
NCHUNK = K // PART
MASK_BIAS = -1.0e6
INV_SQRT_D = 1.0 / math.sqrt(D)
F32 = mybir.dt.float32
F32R = mybir.dt.float32r

# fp32 matmuls stream at 4 cycles/row on the PE; float32r streams at 1
# cycle/row (N>=256) with a reduced-precision multiply. Accuracy was
# validated empirically on HW against the fp32 jax reference.
import os as _os

R_SCORES = _os.environ.get("ATTN_R_SCORES", "1") == "1"
R_AV = _os.environ.get("ATTN_R_AV", "1") == "1"
R_SUMS = _os.environ.get("ATTN_R_SUMS", "1") == "1"
R_RB = _os.environ.get("ATTN_R_RB", "1") == "1"


def _mm(nc, out, lhsT, rhs, fast, **kw):
    if not fast:
        lhsT = lhsT.bitcast(F32)
        rhs = rhs.bitcast(F32)
    nc.tensor.matmul(out, lhsT, rhs, **kw)

_NC_CACHE: dict = {}


def _pack_cols(cap: int):
    """Column layout of the packed per-batch input: qt | kt | vp."""
    kcols = cap * PART
    q0 = 0
    k0 = q0 + Q
    v0 = k0 + kcols
    total = v0 + kcols
    return q0, k0, v0, total


def build_nc(profile: tuple) -> bass.Bass:
    """Build the SPMD Bass program for a per-slot chunk-count profile."""
    nc = bacc.Bacc()
    ins = [
        nc.declare_dram_parameter(
            f"in{s}", [PART, _pack_cols(profile[s])[-1]], F32R, isOutput=False
        )
        for s in range(BPC)
    ]
    mb = nc.declare_dram_parameter("mb", [PART, BPC * NCHUNK], F32, isOutput=False)
    cst = nc.declare_dram_parameter("cst", [PART, PART + 1], F32R, isOutput=False)
    out = nc.declare_dram_parameter("out", [BPC, PART, Q], F32, isOutput=True)
    sums_out = nc.declare_dram_parameter("sums", [BPC, 1, Q], F32, isOutput=True)

    with tile.TileContext(nc) as tc:
        with (
            tc.tile_pool(name="io", bufs=2) as io,
            tc.tile_pool(name="probs", bufs=3) as probs,
            tc.tile_pool(name="consts", bufs=1) as consts,
            tc.tile_pool(name="ps_s", bufs=2, space="PSUM") as ps_s,
            tc.tile_pool(name="ps_out", bufs=1, space="PSUM") as ps_out,
            tc.tile_pool(name="ps_sum", bufs=1, space="PSUM") as ps_sum,
        ):
            cst_sb = consts.tile([PART, PART + 1], F32R)
            nc.sync.dma_start(out=cst_sb, in_=cst[:, :])
            ones_col = cst_sb[:, 0:1]    # lhsT for the denominator
            mb_sb = consts.tile([PART, BPC * NCHUNK], F32)
            nc.sync.dma_start(out=mb_sb, in_=mb[:, :])

            for b in range(BPC):
                cap = profile[b]
                q0, k0, v0, total = _pack_cols(cap)

                in_sb = io.tile([PART, total], F32R, tag=f"in{b}", bufs=1)
                nc.sync.dma_start(out=in_sb, in_=ins[b][:, :])
                qt_sb = in_sb[:, q0:q0 + Q]
                out_ps = ps_out.tile([PART, Q], F32, tag="out")
                sums_ps = ps_sum.tile([1, Q], F32, tag="sums")

                for c in range(cap):
                    s_ps = ps_s.tile([PART, Q], F32, tag="s")
                    kw = in_sb[:, k0 + c * PART:k0 + (c + 1) * PART]
                    for h in range(2):
                        _mm(
                            nc,
                            s_ps[:, h * 512:(h + 1) * 512],
                            kw,
                            qt_sb[:, h * 512:(h + 1) * 512],
                            R_SCORES,
                            start=True,
                            stop=True,
                        )
                    p_sb = probs.tile([PART, Q], F32R, tag="p")
                    nc.scalar.activation(
                        p_sb,
                        s_ps,
                        mybir.ActivationFunctionType.Exp,
                        bias=mb_sb[:, b * NCHUNK + c:b * NCHUNK + c + 1],
                        scale=INV_SQRT_D,
                    )
                    vw = in_sb[:, v0 + c * PART:v0 + (c + 1) * PART]
                    first, last = c == 0, c == cap - 1
                    for h in range(2):
                        _mm(
                            nc,
                            out_ps[:, h * 512:(h + 1) * 512],
                            vw,
                            p_sb[:, h * 512:(h + 1) * 512],
                            R_AV,
                            start=first,
                            stop=last,
                        )
                    for h in range(2):
                        _mm(
                            nc,
                            sums_ps[:, h * 512:(h + 1) * 512],
                            ones_col,
                            p_sb[:, h * 512:(h + 1) * 512],
                            R_SUMS,
                            start=first,
                            stop=last,
                        )

                # Epilogue: PSUM -> SBUF -> DRAM; host divides by sums.
                outn = io.tile([PART, Q], F32, tag="outn")
                nc.scalar.copy(outn, out_ps)
                nc.sync.dma_start(out=out[b], in_=outn)
                sums_sb = probs.tile([1, Q], F32, tag="sums_sb")
                nc.vector.tensor_copy(sums_sb, sums_ps)
                nc.sync.dma_start(out=sums_out[b], in_=sums_sb)

    nc.compile()
    return nc


def plan(valid_lens: np.ndarray):
    """Assign batches to (core, slot) and derive the chunk-count profile.

    Sorting by descending need and slicing slot-major minimizes the sum of
    per-slot maxima, which is the per-core static work.
    """
    need = np.minimum((valid_lens.astype(np.int64) + PART - 1) // PART, NCHUNK)
    need = np.maximum(need, 1)
    order = np.argsort(-need, kind="stable")
    perm = order.reshape(BPC, N_CORES)  # perm[slot, core] = batch index
    profile = tuple(int(need[perm[s]].max()) for s in range(BPC))
    return perm, profile


def kernel(queries, keys, values, valid_lens):
    q = np.ascontiguousarray(np.asarray(queries, dtype=np.float32))
    k = np.ascontiguousarray(np.asarray(keys, dtype=np.float32))
    v = np.ascontiguousarray(np.asarray(values, dtype=np.float32))
    lens = np.asarray(valid_lens).astype(np.int64).reshape(B)

    perm, profile = plan(lens)

    if profile not in _NC_CACHE:
        _NC_CACHE[profile] = build_nc(profile)
    nc = _NC_CACHE[profile]

    arange_k = np.arange(K)
    in_maps = []
    for core in range(N_CORES):
        im = {}
        mb_h = np.empty((PART, BPC * NCHUNK), np.float32)
        for slot in range(BPC):
            cap = profile[slot]
            q0, k0, v0, total = _pack_cols(cap)
            kcols = cap * PART
            bidx = int(perm[slot, core])
            buf = np.empty((PART, total), np.float32)
            buf[:, q0:q0 + Q] = q[bidx].T
            buf[:, k0:k0 + kcols] = k[bidx, :kcols].T
            # v chunk-major: vp[p, c*128 + d] = v[c*128 + p, d]
            buf[:, v0:v0 + kcols] = (
                v[bidx, :kcols]
                .reshape(cap, PART, D)
                .transpose(1, 0, 2)
                .reshape(PART, kcols)
            )
            im[f"in{slot}"] = buf
            # bias[p, c] = 0 if (c*128+p) < L else -1e6
            valid = (arange_k < lens[bidx]).reshape(NCHUNK, PART).T  # [p, c]
            mb_h[:, slot * NCHUNK:(slot + 1) * NCHUNK] = np.where(
                valid, 0.0, MASK_BIAS
            )
        im["mb"] = mb_h
        im["cst"] = np.ones((PART, PART + 1), np.float32)
        in_maps.append(im)

    res = run_bass_kernel_spmd(nc, in_maps, list(range(N_CORES)))

    out = np.empty((B, Q, D), np.float32)
    for core in range(N_CORES):
        core_out = res.results[core]["out"]   # [BPC, 128(v), 1024(q)]
        core_sums = res.results[core]["sums"]  # [BPC, 1, 1024(q)]
        for slot in range(BPC):
            bidx = int(perm[slot, core])
            out[bidx] = (core_out[slot] / core_sums[slot]).T
    return out


# revision 18
# speedup vs baseline: 3.2139x; 1.1237x over previous
"""Masked dot-product attention on 8 Trainium2 NeuronCores (Bass/Tile).

Problem: queries/keys/values [32, 1024, 128] f32, valid_lens [32] i32.
  out = softmax(mask(Q K^T / sqrt(128))) V        (key-padding prefix mask)

Strategy (batch-parallel, 4 batches per core, one SPMD program):
  * Host pre-transposes Q and K per batch to [D=128, 1024] so the
    contraction dim D sits on SBUF partitions; no on-device transposes.
  * Scores are computed transposed: S^T[k, q] = (K^T chunk).T @ Q^T with k
    in chunks of 128 partitions.
  * The prefix key mask is per-PARTITION in this layout, so it folds into
    the exp for free: ACT computes exp(S^T * 1/sqrt(D) + bias) with
    bias[k] in {0, -1e6}; masked rows become exactly 0.
  * out^T[v, q] += V_chunk-as-lhsT @ expS^T accumulates in PSUM across
    k chunks (V is loaded chunk-major, no transpose needed).
  * denominator[q] = ones-column matmuls on the same expS^T chunks,
    accumulated in PSUM (exact: multiply by 1.0).
  * out^T and sums are DMA'd back; the host divides and transposes
    while gathering (0.003% of the FLOPs).
  * float32r everywhere on the PE: 1 cycle/row instead of fp32's 4.

Static masked-chunk skipping: batch b only needs ceil(valid_lens[b]/128)
key chunks; the rest contribute exactly 0. Batches are assigned to the 4
per-core slots by descending need (sorted, slot-major), so slot j's
compile-time chunk count is max over its 8 batches. The SPMD program is
specialized to that profile at kernel build time.

The chunk loop is software-pipelined: chunk c+1's score matmuls are
emitted before chunk c's AV/sums matmuls so the PE produces the next
exp's input first and ACT never starves.
"""

import math
import os as _os

import numpy as np

import concourse.bacc as bacc
import concourse.bass as bass
import concourse.mybir as mybir
import concourse.tile as tile
from concourse.bass_utils import run_bass_kernel_spmd

B, Q, K, D = 32, 1024, 1024, 128
N_CORES = 8
BPC = B // N_CORES  # batches per core
PART = 128          # partition size / key chunk size
NCHUNK = K // PART
MASK_BIAS = -1.0e6
INV_SQRT_D = 1.0 / math.sqrt(D)
F32 = mybir.dt.float32
F32R = mybir.dt.float32r

_NC_CACHE: dict = {}


def build_nc(profile: tuple) -> bass.Bass:
    """Build the SPMD Bass program for a per-slot chunk-count profile."""
    nc = bacc.Bacc()
    qt = nc.declare_dram_parameter("qt", [BPC, PART, Q], F32R, isOutput=False)
    kt = nc.declare_dram_parameter("kt", [BPC, PART, K], F32R, isOutput=False)
    vp = nc.declare_dram_parameter("vp", [BPC, PART, K], F32R, isOutput=False)
    mb = nc.declare_dram_parameter("mb", [PART, BPC * NCHUNK], F32, isOutput=False)
    cst = nc.declare_dram_parameter("cst", [PART, 1], F32R, isOutput=False)
    out = nc.declare_dram_parameter("out", [BPC, PART, Q], F32, isOutput=True)
    sums_out = nc.declare_dram_parameter("sums", [BPC, 1, Q], F32, isOutput=True)

    with tile.TileContext(nc) as tc:
        with (
            tc.tile_pool(name="io", bufs=2) as io,
            tc.tile_pool(name="probs", bufs=4) as probs,
            tc.tile_pool(name="consts", bufs=1) as consts,
            tc.tile_pool(name="ps_s", bufs=2, space="PSUM") as ps_s,
            tc.tile_pool(name="ps_acc", bufs=1, space="PSUM") as ps_acc,
        ):
            ones_col = consts.tile([PART, 1], F32R)  # lhsT for the denominator
            nc.sync.dma_start(out=ones_col, in_=cst[:, :])
            mb_sb = consts.tile([PART, BPC * NCHUNK], F32)
            nc.sync.dma_start(out=mb_sb, in_=mb[:, :])

            for b in range(BPC):
                cap = profile[b]
                kcols = cap * PART

                qt_sb = io.tile([PART, Q], F32R, tag="qt")
                nc.sync.dma_start(out=qt_sb, in_=qt[b])
                kt_sb = io.tile([PART, kcols], F32R, tag="kt")
                nc.sync.dma_start(out=kt_sb, in_=kt[b][:, :kcols])
                vp_sb = io.tile([PART, kcols], F32R, tag="vp")
                nc.sync.dma_start(out=vp_sb, in_=vp[b][:, :kcols])

                out_ps = ps_acc.tile([PART, Q], F32, tag="out")
                sums_ps = ps_acc.tile([1, Q], F32, tag="sums")

                def s_mms(c):
                    s_ps = ps_s.tile([PART, Q], F32, tag="s", name=f"s_b{b}c{c}")
                    kw = kt_sb[:, c * PART:(c + 1) * PART]
                    for h in range(2):
                        nc.tensor.matmul(
                            s_ps[:, h * 512:(h + 1) * 512],
                            kw,
                            qt_sb[:, h * 512:(h + 1) * 512],
                            start=True,
                            stop=True,
                        )
                    return s_ps

                s_cur = s_mms(0)
                for c in range(cap):
                    p_sb = probs.tile([PART, Q], F32R, tag="p")
                    nc.scalar.activation(
                        p_sb,
                        s_cur,
                        mybir.ActivationFunctionType.Exp,
                        bias=mb_sb[:, b * NCHUNK + c:b * NCHUNK + c + 1],
                        scale=INV_SQRT_D,
                    )
                    if c + 1 < cap:
                        s_cur = s_mms(c + 1)
                    vw = vp_sb[:, c * PART:(c + 1) * PART]
                    first, last = c == 0, c == cap - 1
                    for h in range(2):
                        nc.tensor.matmul(
                            out_ps[:, h * 512:(h + 1) * 512],
                            vw,
                            p_sb[:, h * 512:(h + 1) * 512],
                            start=first,
                            stop=last,
                        )
                    for h in range(2):
                        nc.tensor.matmul(
                            sums_ps[:, h * 512:(h + 1) * 512],
                            ones_col[:, :],
                            p_sb[:, h * 512:(h + 1) * 512],
                            start=first,
                            stop=last,
                        )

                # Epilogue: PSUM -> SBUF (split ACT/DVE) -> DRAM; host divides.
                outn = io.tile([PART, Q], F32, tag="outn")
                nc.scalar.copy(outn[:, 0:512], out_ps[:, 0:512])
                nc.vector.tensor_copy(outn[:, 512:1024], out_ps[:, 512:1024])
                nc.sync.dma_start(out=out[b], in_=outn)
                sums_sb = probs.tile([1, Q], F32, tag="sums_sb")
                nc.vector.tensor_copy(sums_sb, sums_ps)
                nc.sync.dma_start(out=sums_out[b], in_=sums_sb)

    nc.compile()
    return nc


def plan(valid_lens: np.ndarray):
    """Assign batches to (core, slot) and derive the chunk-count profile.

    Sorting by descending need and slicing slot-major minimizes the sum of
    per-slot maxima, which is the per-core static work.
    """
    need = np.minimum((valid_lens.astype(np.int64) + PART - 1) // PART, NCHUNK)
    need = np.maximum(need, 1)
    order = np.argsort(-need, kind="stable")
    perm = order.reshape(BPC, N_CORES)  # perm[slot, core] = batch index
    profile = tuple(int(need[perm[s]].max()) for s in range(BPC))
    return perm, profile


def kernel(queries, keys, values, valid_lens):
    q = np.ascontiguousarray(np.asarray(queries, dtype=np.float32))
    k = np.ascontiguousarray(np.asarray(keys, dtype=np.float32))
    v = np.ascontiguousarray(np.asarray(values, dtype=np.float32))
    lens = np.asarray(valid_lens).astype(np.int64).reshape(B)

    perm, profile = plan(lens)

    if profile not in _NC_CACHE:
        _NC_CACHE[profile] = build_nc(profile)
    nc = _NC_CACHE[profile]

    # Vectorized host layout prep: obi[core, slot] = batch index.
    obi = perm.T  # [N_CORES, BPC]
    qt_all = np.ascontiguousarray(q[obi].transpose(0, 1, 3, 2))  # [8,4,128,1024]
    kt_all = np.ascontiguousarray(k[obi].transpose(0, 1, 3, 2))
    # v chunk-major: vp[p, c*128 + d] = v[c*128 + p, d]
    vp_all = np.ascontiguousarray(
        v[obi]
        .reshape(N_CORES, BPC, NCHUNK, PART, D)
        .transpose(0, 1, 3, 2, 4)
        .reshape(N_CORES, BPC, PART, K)
    )
    # bias[p, slot*8 + c] = 0 if (c*128+p) < L else -1e6
    valid = np.arange(K)[None, None, :] < lens[obi][:, :, None]  # [8,4,1024]
    mb_all = np.where(
        valid.reshape(N_CORES, BPC, NCHUNK, PART).transpose(0, 2, 3, 1), 0.0, MASK_BIAS
    ).astype(np.float32)  # [8, NCHUNK, PART, BPC] -> need [8, PART, BPC*NCHUNK]
    mb_all = np.ascontiguousarray(
        mb_all.transpose(0, 2, 3, 1).reshape(N_CORES, PART, BPC * NCHUNK)
    )
    ones = np.ones((PART, 1), np.float32)

    in_maps = [
        {
            "qt": qt_all[core],
            "kt": kt_all[core],
            "vp": vp_all[core],
            "mb": mb_all[core],
            "cst": ones,
        }
        for core in range(N_CORES)
    ]

    res = run_bass_kernel_spmd(nc, in_maps, list(range(N_CORES)))

    out = np.empty((B, Q, D), np.float32)
    for core in range(N_CORES):
        core_out = res.results[core]["out"]    # [BPC, 128(v), 1024(q)]
        core_sums = res.results[core]["sums"]  # [BPC, 1, 1024(q)]
        for slot in range(BPC):
            bidx = int(perm[slot, core])
            out[bidx] = (core_out[slot] / core_sums[slot]).T
    return out


# revision 19
# speedup vs baseline: 3.2831x; 1.0215x over previous
"""Masked dot-product attention on 8 Trainium2 NeuronCores (Bass/Tile).

Problem: queries/keys/values [32, 1024, 128] f32, valid_lens [32] i32.
  out = softmax(mask(Q K^T / sqrt(128))) V        (key-padding prefix mask)

Strategy (batch-parallel, 4 batches per core, one SPMD program):
  * Host pre-transposes Q and K per batch to [D=128, 1024] so the
    contraction dim D sits on SBUF partitions; no on-device transposes.
  * Scores are computed transposed: S^T[k, q] = (K^T chunk).T @ Q^T with k
    in chunks of 128 partitions.
  * The prefix key mask is per-PARTITION in this layout, so it folds into
    the exp for free: ACT computes exp(S^T * 1/sqrt(D) + bias) with
    bias[k] in {0, -1e6}; masked rows become exactly 0.
  * out^T[v, q] += V_chunk-as-lhsT @ expS^T accumulates in PSUM across
    k chunks (V is loaded chunk-major, no transpose needed).
  * denominator[q] = ones-column matmuls on the same expS^T chunks,
    accumulated in PSUM (exact: multiply by 1.0).
  * out^T and sums are DMA'd back; the host divides and transposes
    while gathering (0.003% of the FLOPs).
  * float32r everywhere on the PE: 1 cycle/row instead of fp32's 4.

Static masked-chunk skipping: batch b only needs ceil(valid_lens[b]/128)
key chunks; the rest contribute exactly 0. Batches are assigned to the 4
per-core slots by descending need (sorted, slot-major), so slot j's
compile-time chunk count is max over its 8 batches. The SPMD program is
specialized to that profile at kernel build time.

The chunk loop is software-pipelined: chunk c+1's score matmuls are
emitted before chunk c's AV/sums matmuls so the PE produces the next
exp's input first and ACT never starves.
"""

import math
import os as _os

import numpy as np

import concourse.bacc as bacc
import concourse.bass as bass
import concourse.mybir as mybir
import concourse.tile as tile
from concourse.bass_utils import run_bass_kernel_spmd

B, Q, K, D = 32, 1024, 1024, 128
N_CORES = 8
BPC = B // N_CORES  # batches per core
PART = 128          # partition size / key chunk size
NCHUNK = K // PART
MASK_BIAS = -1.0e6
INV_SQRT_D = 1.0 / math.sqrt(D)
F32 = mybir.dt.float32
F32R = mybir.dt.float32r

_NC_CACHE: dict = {}


def build_nc(profile: tuple) -> bass.Bass:
    """Build the SPMD Bass program for a per-slot chunk-count profile."""
    nc = bacc.Bacc()
    qt = nc.declare_dram_parameter("qt", [BPC, PART, Q], F32R, isOutput=False)
    kt = nc.declare_dram_parameter("kt", [BPC, PART, K], F32R, isOutput=False)
    vp = nc.declare_dram_parameter("vp", [BPC, PART, K], F32R, isOutput=False)
    mb = nc.declare_dram_parameter("mb", [PART, BPC * NCHUNK], F32, isOutput=False)
    cst = nc.declare_dram_parameter("cst", [PART, 1], F32R, isOutput=False)
    out = nc.declare_dram_parameter("out", [BPC, PART, Q], F32, isOutput=True)
    sums_out = nc.declare_dram_parameter("sums", [BPC, 1, Q], F32, isOutput=True)

    with tile.TileContext(nc) as tc:
        with (
            tc.tile_pool(name="io", bufs=2) as io,
            tc.tile_pool(name="probs", bufs=8) as probs,
            tc.tile_pool(name="consts", bufs=1) as consts,
            tc.tile_pool(name="ps_s", bufs=2, space="PSUM") as ps_s,
            tc.tile_pool(name="ps_acc", bufs=1, space="PSUM") as ps_acc,
        ):
            # Startup-ordered loads: batch 0's operands first (SP HWDGE ring
            # is FIFO), then the small consts, then the rest.
            ins_sb = []
            for b in range(BPC):
                cap = profile[b]
                kcols = cap * PART
                qt_sb = io.tile([PART, Q], F32R, tag="qt", name=f"qt{b}")
                kt_sb = io.tile([PART, kcols], F32R, tag="kt", name=f"kt{b}")
                vp_sb = io.tile([PART, kcols], F32R, tag="vp", name=f"vp{b}")
                ins_sb.append((qt_sb, kt_sb, vp_sb))
                nc.sync.dma_start(out=kt_sb, in_=kt[b][:, :kcols])
                nc.sync.dma_start(out=qt_sb, in_=qt[b])
                if b == 0:
                    ones_col = consts.tile([PART, 1], F32R)
                    nc.sync.dma_start(out=ones_col, in_=cst[:, :])
                    mb_sb = consts.tile([PART, BPC * NCHUNK], F32)
                    nc.sync.dma_start(out=mb_sb, in_=mb[:, :])
                nc.sync.dma_start(out=vp_sb, in_=vp[b][:, :kcols])

            for b in range(BPC):
                cap = profile[b]
                qt_sb, kt_sb, vp_sb = ins_sb[b]

                out_ps = ps_acc.tile([PART, Q], F32, tag="out")
                sums_ps = ps_acc.tile([1, Q], F32, tag="sums")

                def s_mms(c):
                    s_ps = ps_s.tile([PART, Q], F32, tag="s", name=f"s_b{b}c{c}")
                    kw = kt_sb[:, c * PART:(c + 1) * PART]
                    for h in range(2):
                        nc.tensor.matmul(
                            s_ps[:, h * 512:(h + 1) * 512],
                            kw,
                            qt_sb[:, h * 512:(h + 1) * 512],
                            start=True,
                            stop=True,
                        )
                    return s_ps

                s_cur = s_mms(0)
                for c in range(cap):
                    p_sb = probs.tile([PART, Q], F32R, tag="p")
                    nc.scalar.activation(
                        p_sb,
                        s_cur,
                        mybir.ActivationFunctionType.Exp,
                        bias=mb_sb[:, b * NCHUNK + c:b * NCHUNK + c + 1],
                        scale=INV_SQRT_D,
                    )
                    if c + 1 < cap:
                        s_cur = s_mms(c + 1)
                    vw = vp_sb[:, c * PART:(c + 1) * PART]
                    first, last = c == 0, c == cap - 1
                    for h in range(2):
                        nc.tensor.matmul(
                            out_ps[:, h * 512:(h + 1) * 512],
                            vw,
                            p_sb[:, h * 512:(h + 1) * 512],
                            start=first,
                            stop=last,
                        )
                    for h in range(2):
                        nc.tensor.matmul(
                            sums_ps[:, h * 512:(h + 1) * 512],
                            ones_col[:, :],
                            p_sb[:, h * 512:(h + 1) * 512],
                            start=first,
                            stop=last,
                        )

                # Epilogue: PSUM -> SBUF (split ACT/DVE) -> DRAM; host divides.
                outn = io.tile([PART, Q], F32, tag="outn")
                nc.scalar.copy(outn[:, 0:512], out_ps[:, 0:512])
                nc.vector.tensor_copy(outn[:, 512:1024], out_ps[:, 512:1024])
                nc.sync.dma_start(out=out[b], in_=outn)
                sums_sb = probs.tile([1, Q], F32, tag="sums_sb")
                nc.vector.tensor_copy(sums_sb, sums_ps)
                nc.sync.dma_start(out=sums_out[b], in_=sums_sb)

    nc.compile()
    return nc


def plan(valid_lens: np.ndarray):
    """Assign batches to (core, slot) and derive the chunk-count profile.

    Sorting by descending need and slicing slot-major minimizes the sum of
    per-slot maxima, which is the per-core static work.
    """
    need = np.minimum((valid_lens.astype(np.int64) + PART - 1) // PART, NCHUNK)
    need = np.maximum(need, 1)
    order = np.argsort(-need, kind="stable")
    perm = order.reshape(BPC, N_CORES)  # perm[slot, core] = batch index
    profile = tuple(int(need[perm[s]].max()) for s in range(BPC))
    return perm, profile


def kernel(queries, keys, values, valid_lens):
    q = np.ascontiguousarray(np.asarray(queries, dtype=np.float32))
    k = np.ascontiguousarray(np.asarray(keys, dtype=np.float32))
    v = np.ascontiguousarray(np.asarray(values, dtype=np.float32))
    lens = np.asarray(valid_lens).astype(np.int64).reshape(B)

    perm, profile = plan(lens)

    if profile not in _NC_CACHE:
        _NC_CACHE[profile] = build_nc(profile)
    nc = _NC_CACHE[profile]

    # Vectorized host layout prep: obi[core, slot] = batch index.
    obi = perm.T  # [N_CORES, BPC]
    qt_all = np.ascontiguousarray(q[obi].transpose(0, 1, 3, 2))  # [8,4,128,1024]
    kt_all = np.ascontiguousarray(k[obi].transpose(0, 1, 3, 2))
    # v chunk-major: vp[p, c*128 + d] = v[c*128 + p, d]
    vp_all = np.ascontiguousarray(
        v[obi]
        .reshape(N_CORES, BPC, NCHUNK, PART, D)
        .transpose(0, 1, 3, 2, 4)
        .reshape(N_CORES, BPC, PART, K)
    )
    # bias[p, slot*8 + c] = 0 if (c*128+p) < L else -1e6
    valid = np.arange(K)[None, None, :] < lens[obi][:, :, None]  # [8,4,1024]
    mb_all = np.where(
        valid.reshape(N_CORES, BPC, NCHUNK, PART).transpose(0, 2, 3, 1), 0.0, MASK_BIAS
    ).astype(np.float32)  # [8, NCHUNK, PART, BPC] -> need [8, PART, BPC*NCHUNK]
    mb_all = np.ascontiguousarray(
        mb_all.transpose(0, 2, 3, 1).reshape(N_CORES, PART, BPC * NCHUNK)
    )
    ones = np.ones((PART, 1), np.float32)

    in_maps = [
        {
            "qt": qt_all[core],
            "kt": kt_all[core],
            "vp": vp_all[core],
            "mb": mb_all[core],
            "cst": ones,
        }
        for core in range(N_CORES)
    ]

    res = run_bass_kernel_spmd(nc, in_maps, list(range(N_CORES)))

    out = np.empty((B, Q, D), np.float32)
    for core in range(N_CORES):
        core_out = res.results[core]["out"]    # [BPC, 128(v), 1024(q)]
        core_sums = res.results[core]["sums"]  # [BPC, 1, 1024(q)]
        for slot in range(BPC):
            bidx = int(perm[slot, core])
            out[bidx] = (core_out[slot] / core_sums[slot]).T
    return out
